# revision 1
# baseline (speedup 1.0000x reference)
"""RNN-T JointNet kernel for 8 Trainium2 NeuronCores.

Math: out[b,t,u,:] = gelu_tanh(concat(enc[b,t], dec[b,u])) @ W_fc^T + b_fc
Since gelu is elementwise, gelu(concat(a,b)) = concat(gelu(a), gelu(b)), so
  out[b,t,u,:] = P_enc[b,t,:] + P_dec[b,u,:]
with P_enc = gelu(enc) @ W_fc[:, :512]^T + b_fc  (tiny matmul, (B,T,V))
     P_dec = gelu(dec) @ W_fc[:, 512:]^T          (tiny matmul, (B,U,V))
The dominant cost is streaming the (B,T,U,V) = 310MB output to HBM.

Sharding: 8 cores = 4 batches x 2 u-halves. Core c -> b = c//2, u-range
[ (c%2)*52, (c%2)*52+52 ) of U padded 101->104 (pad rows are zeros and
trimmed on gather). Full T=300 per core. This halves the per-core count
of PE broadcast matmuls (the previous critical path) vs t-sharding.

Device pipeline per core:
  1. gelu(enc), gelu(dec_slice) on ACT; PE-transpose to [d, t] layout.
  2. PE matmuls -> P_enc [300,640] (bias folded via K=1 ones matmul),
     P_dec [52,640]; P_dec bounced through DRAM to a partition-0 row tile.
  3. Loop over u-pairs: PE broadcasts 2 P_dec rows across 128 partitions
     (K=1 matmuls, 512-aligned chunks into one PSUM tile), ACT copies
     PSUM->SBUF, DVE adds P_enc for t-chunks 0/1 (SBUF+SBUF), gpsimd adds
     the 44-row t-tail. Per 4-u block, 3 strided DMAs (~1.3MB) store to
     DRAM, alternating between the two HWDGE rings (sync/scalar).
"""

import numpy as np

B, T, U = 4, 300, 101
D = 512
V = 640
UCORE = 52  # u rows per core (U padded to 104)
NCORES = 8

LAST_RESULT = None  # BassKernelResults of the most recent run (for test.py)
RUN_KWARGS = {}  # extra kwargs test.py may inject (e.g. tmpdir for traces)

_cache = {}


def _build():
    import concourse.mybir as mybir
    from concourse import bacc, masks
    from concourse.tile import TileContext

    f32 = mybir.dt.float32
    AF = mybir.ActivationFunctionType

    nc = bacc.Bacc()
    enc_d = nc.dram_tensor("enc", [T, D], f32, kind="ExternalInput")
    dec_d = nc.dram_tensor("dec", [UCORE, D], f32, kind="ExternalInput")
    wT_d = nc.dram_tensor("wT", [2 * D, V], f32, kind="ExternalInput")
    bias_d = nc.dram_tensor("bias", [1, V], f32, kind="ExternalInput")
    out_d = nc.dram_tensor("out", [T, UCORE, V], f32, kind="ExternalOutput")

    tchunks = [(0, 128), (128, 128), (256, 44)]
    vchunks = [(0, 512), (512, V - 512)]

    with TileContext(nc) as tc:
        with (
            tc.tile_pool(name="const", bufs=1) as constp,
            tc.tile_pool(name="work", bufs=2) as work,
            tc.tile_pool(name="persist", bufs=1) as persist,
            tc.tile_pool(name="outp0", bufs=4) as outp0,
            tc.tile_pool(name="outp1", bufs=2) as outp1,
            tc.tile_pool(name="outp2", bufs=2) as outp2,
            tc.tile_pool(name="bcp", bufs=4) as bcp,
            tc.tile_pool(name="rowp", bufs=3) as rowp,
            tc.tile_pool(name="dramp", bufs=1, space="DRAM") as dramp,
            tc.tile_pool(name="pre_psum", bufs=1, space="PSUM") as pre_psum,
            tc.tile_pool(name="loop_psum", bufs=2, space="PSUM") as loop_psum,
        ):
            ident = constp.tile([128, 128], f32)
            masks.make_identity(nc, ident[:])
            ones = constp.tile([1, 128], f32)
            nc.gpsimd.memset(ones[:], 1.0)
            bias_sb = constp.tile([1, V], f32)
            nc.sync.dma_start(bias_sb[:], bias_d[:])

            # dummy PE op: absorbs the gpsimd-sem wait once so later
            # transposes/matmuls never carry >1 wait (S3_LW slot limit)
            warm = pre_psum.tile([128, 128], f32, tag="tr")
            nc.tensor.transpose(warm[:32, :32], ident[:32, :32], ident[:32, :32])

            # weights: [1024 -> 8 chunks of 128 on partitions, 640 free]
            w_sb = []
            for kc in range(8):
                wt = persist.tile([128, V], f32, tag=f"w{kc}", name=f"w{kc}")
                nc.sync.dma_start(wt[:], wT_d[kc * 128 : (kc + 1) * 128, :])
                w_sb.append(wt)

            # gelu(enc), gelu(dec)
            genc = []
            for i, (t0, tn) in enumerate(tchunks):
                et = work.tile([128, D], f32, tag="ld_in", name="et")
                nc.sync.dma_start(et[:tn, :], enc_d[t0 : t0 + tn, :])
                gt = persist.tile([128, D], f32, tag=f"genc{i}", name=f"genc{i}")
                nc.scalar.activation(gt[:tn, :], et[:tn, :], AF.Gelu_apprx_tanh)
                genc.append(gt)
            dt_in = work.tile([128, D], f32, tag="ld_in")
            nc.sync.dma_start(dt_in[:UCORE, :], dec_d[:, :])
            gdec = persist.tile([128, D], f32, tag="gdec")
            nc.scalar.activation(gdec[:UCORE, :], dt_in[:UCORE, :], AF.Gelu_apprx_tanh)

            # transpose to [d, t] / [d, u]
            gencT = [persist.tile([128, T], f32, tag=f"gencT{d}", name=f"gencT{d}") for d in range(4)]
            gdecT = [persist.tile([128, UCORE], f32, tag=f"gdecT{d}", name=f"gdecT{d}") for d in range(4)]
            for dch in range(4):
                dsl = slice(dch * 128, (dch + 1) * 128)
                for i, (t0, tn) in enumerate(tchunks):
                    ps = pre_psum.tile([128, 128], f32, tag="tr")
                    nc.tensor.transpose(ps[:, :tn], genc[i][:tn, dsl], ident[:tn, :tn])
                    nc.scalar.copy(gencT[dch][:, t0 : t0 + tn], ps[:, :tn])
                ps = pre_psum.tile([128, 128], f32, tag="tr")
                nc.tensor.transpose(ps[:, :UCORE], gdec[:UCORE, dsl], ident[:UCORE, :UCORE])
                nc.scalar.copy(gdecT[dch][:, :UCORE], ps[:, :UCORE])

            # P_enc (with bias), P_dec
            pe_sb = [persist.tile([128, V], f32, tag=f"pe{i}", name=f"pe{i}") for i in range(3)]
            pd_sb = persist.tile([128, V], f32, tag="pd")
            for i, (t0, tn) in enumerate(tchunks):
                for v0, vn in vchunks:
                    ps = pre_psum.tile([128, 512], f32, tag="mm")
                    for d in range(4):
                        nc.tensor.matmul(
                            ps[:tn, :vn],
                            gencT[d][:, t0 : t0 + tn],
                            w_sb[d][:, v0 : v0 + vn],
                            start=(d == 0),
                            stop=False,
                        )
                    nc.tensor.matmul(
                        ps[:tn, :vn],
                        ones[:1, :tn],
                        bias_sb[:1, v0 : v0 + vn],
                        start=False,
                        stop=True,
                    )
                    nc.scalar.copy(pe_sb[i][:tn, v0 : v0 + vn], ps[:tn, :vn])
            for v0, vn in vchunks:
                ps = pre_psum.tile([128, 512], f32, tag="mm")
                for d in range(4):
                    nc.tensor.matmul(
                        ps[:UCORE, :vn],
                        gdecT[d][:, :UCORE],
                        w_sb[4 + d][:, v0 : v0 + vn],
                        start=(d == 0),
                        stop=(d == 3),
                    )
                nc.scalar.copy(pd_sb[:UCORE, v0 : v0 + vn], ps[:UCORE, :vn])

            # bounce P_dec through DRAM so rows land on partition 0
            pd_dram = dramp.tile([UCORE, V], f32)
            nc.sync.dma_start(pd_dram[:, :], pd_sb[:UCORE, :])

            UB = 4  # u rows per store block (52 = 13 * 4)
            for bi, u0 in enumerate(range(0, UCORE, UB)):
                rows = rowp.tile([1, UB * V], f32, tag="rows")
                nc.sync.dma_start(rows[:1, :], pd_dram[u0 : u0 + UB, :])
                ots = [
                    outp0.tile([128, UB, V], f32, tag="ot0", name="ot0"),
                    outp1.tile([128, UB, V], f32, tag="ot1", name="ot1"),
                    outp2.tile([128, UB, V], f32, tag="ot2", name="ot2"),
                ]
                for j0 in (0, 2):
                    fl = 2 * V  # 1280 flat elems per pair
                    ps = loop_psum.tile([128, 2 * V], f32, tag="bc")
                    for c0 in range(0, fl, 512):
                        cn = min(512, fl - c0)
                        nc.tensor.matmul(
                            ps[:, c0 : c0 + cn],
                            ones[:1, :128],
                            rows[:1, j0 * V + c0 : j0 * V + c0 + cn],
                            start=True,
                            stop=True,
                        )
                    bc = bcp.tile([128, 2 * V], f32, tag="bc_sb")
                    nc.scalar.copy(bc[:, :], ps[:, :])
                    for l in range(2):
                        j = j0 + l
                        bcv = bc[:, l * V : (l + 1) * V]
                        nc.vector.tensor_add(ots[0][:, j, :], pe_sb[0][:, :], bcv)
                        nc.vector.tensor_add(ots[1][:, j, :], pe_sb[1][:, :], bcv)
                        nc.gpsimd.tensor_add(
                            ots[2][:44, j, :], pe_sb[2][:44, :], bc[:44, l * V : (l + 1) * V]
                        )
                engs = (
                    (nc.sync, nc.scalar, nc.sync)
                    if bi % 2 == 0
                    else (nc.scalar, nc.sync, nc.scalar)
                )
                for (t0, tn), ot, eng in zip(tchunks, ots, engs):
                    eng.dma_start(out_d[t0 : t0 + tn, u0 : u0 + UB, :], ot[:tn, :, :])

    nc.compile()
    return nc


def kernel(encoder_outputs, decoder_outputs, W_fc, b_fc):
    global LAST_RESULT
    import os

    from concourse.bass_utils import run_bass_kernel_spmd

    enc = np.ascontiguousarray(encoder_outputs, dtype=np.float32)
    dec = np.ascontiguousarray(decoder_outputs, dtype=np.float32)
    wT = np.ascontiguousarray(np.asarray(W_fc, dtype=np.float32).T)  # (1024, 640)
    bias = np.ascontiguousarray(np.asarray(b_fc, dtype=np.float32)[None, :])

    dec_pad = np.zeros((B, 2 * UCORE, D), dtype=np.float32)
    dec_pad[:, :U, :] = dec

    if "nc" not in _cache:
        _cache["nc"] = _build()
    nc = _cache["nc"]

    in_maps = []
    for c in range(NCORES):
        b, uh = c // 2, c % 2
        in_maps.append(
            {
                "enc": np.ascontiguousarray(enc[b]),
                "dec": np.ascontiguousarray(dec_pad[b, uh * UCORE : (uh + 1) * UCORE]),
                "wT": wT,
                "bias": bias,
            }
        )

    res = run_bass_kernel_spmd(
        nc,
        in_maps,
        list(range(NCORES)),
        trace=bool(int(os.environ.get("KJ_TRACE", "0"))),
        **RUN_KWARGS,
    )
    LAST_RESULT = res

    out = np.empty((B, T, U, V), dtype=np.float32)
    for c in range(NCORES):
        b, uh = c // 2, c % 2
        cut = res.results[c]["out"]  # (300, 52, 640)
        if uh == 0:
            out[b, :, :UCORE] = cut
        else:
            out[b, :, UCORE:U] = cut[:, : U - UCORE]
    return out



# revision 3
# speedup vs baseline: 1.7154x; 1.7154x over previous
"""RNN-T JointNet kernel for 8 Trainium2 NeuronCores.

Math: out[b,t,u,:] = gelu_tanh(concat(enc[b,t], dec[b,u])) @ W_fc^T + b_fc
Since gelu is elementwise, gelu(concat(a,b)) = concat(gelu(a), gelu(b)), so
  out[b,t,u,:] = P_enc[b,t,:] + P_dec[b,u,:]
with P_enc = gelu(enc) @ W_fc[:, :512]^T + b_fc  (small matmul, (B,T,V))
     P_dec = gelu(dec) @ W_fc[:, 512:]^T         (small matmul, (B,U,V))
The dominant cost is streaming the (B,T,U,V) = 310MB f32 output to HBM
(~111us/core at 358 GB/s); everything else must hide under the stores.

Sharding: 8 cores = 4 batches x 2 u-halves. Core c -> b = c//2, u-range
[(c%2)*52, (c%2)*52+52) of U padded 101->104. Full T=300 per core.

vs the first version (241.8us): inputs/weights are cast to bf16 on the host
(bf16 matmuls are 4x faster on PE and halve the weight DMA), P_dec rows are
staged ONCE into a partition-{0,32,64} row tile (no per-block row DMAs
trapped behind stores in the HWDGE FIFO), the DVE adds read the broadcast
P_dec tile straight from PSUM with a stride-0 AP over the two 128-row
t-chunks (one DVE op per u, no per-u ACT copy), and the 44-row t-tail is
one big gpsimd add per 4-u block. Per block (4 u): DVE 4x1280-elem adds
(~6.7us), one 2.62MB + one 0.45MB store alternating the two HWDGE rings
(8.6us, the wall), PE/ACT/gpsimd well under.
"""

import numpy as np

B, T, U = 4, 300, 101
D = 512
V = 640
UCORE = 52  # u rows per core (U padded to 104)
NCORES = 8
UB = 4  # u rows per store block (52 = 13 * 4)
RG = 18  # u rows per row-group partition (groups at partitions 0/32/64)

LAST_RESULT = None  # BassKernelResults of the most recent run (for test.py)
RUN_KWARGS = {}  # extra kwargs test.py may inject (e.g. tmpdir for traces)

_cache = {}


def _build():
    import concourse.mybir as mybir
    from concourse import bacc, masks
    from concourse.tile import TileContext

    f32 = mybir.dt.float32
    bf16 = mybir.dt.bfloat16
    AF = mybir.ActivationFunctionType

    nc = bacc.Bacc()
    enc_d = nc.dram_tensor("enc", [T, D], bf16, kind="ExternalInput")
    dec_d = nc.dram_tensor("dec", [UCORE, D], bf16, kind="ExternalInput")
    wT_d = nc.dram_tensor("wT", [2 * D, V], bf16, kind="ExternalInput")
    bias_d = nc.dram_tensor("bias", [1, V], bf16, kind="ExternalInput")
    out_d = nc.dram_tensor("out", [T, UCORE, V], f32, kind="ExternalOutput")

    tchunks = [(0, 128), (128, 128), (256, 44)]
    vchunks = [(0, 512), (512, V - 512)]

    with TileContext(nc) as tc:
        with (
            tc.tile_pool(name="const", bufs=1) as constp,
            tc.tile_pool(name="work", bufs=2) as work,
            tc.tile_pool(name="persist", bufs=1) as persist,
            tc.tile_pool(name="outpA", bufs=3) as outpA,
            tc.tile_pool(name="outpB", bufs=3) as outpB,
            tc.tile_pool(name="bctp", bufs=3) as bctp,
            tc.tile_pool(name="dramp", bufs=1, space="DRAM") as dramp,
            tc.tile_pool(name="psum", bufs=1, space="PSUM") as psum,
        ):
            ident = constp.tile([128, 128], bf16)
            masks.make_identity(nc, ident[:])
            # ones at base partitions 0/32/64 (matmul lhsT/rhs must share base)
            ones3 = constp.tile([65, 128], bf16)
            nc.gpsimd.memset(ones3[:], 1.0)
            bias_sb = constp.tile([1, V], bf16)
            nc.scalar.dma_start(bias_sb[:], bias_d[:])

            # dummy PE op: absorbs the gpsimd-sem wait once so later
            # transposes/matmuls never carry >1 wait (S3_LW slot limit)
            warm = psum.tile([128, 128], bf16, tag="tr", bufs=2)
            nc.tensor.transpose(warm[:32, :32], ident[:32, :32], ident[:32, :32])

            # weights: one DMA into [128 part, 8 k-chunks, 640] bf16
            w_bf = persist.tile([128, 8, V], bf16, tag="w")
            nc.sync.dma_start(w_bf[:, :, :], wT_d[:, :].rearrange("(c p) v -> p c v", c=8))

            # loads + gelu (dec first: it heads the deeper P_dec->rows chain)
            dt_in = work.tile([128, D], bf16, tag="ld", name="dt_in")
            nc.scalar.dma_start(dt_in[:UCORE, :], dec_d[:, :])
            gdec = persist.tile([128, D], bf16, tag="gdec")
            nc.scalar.activation(gdec[:UCORE, :], dt_in[:UCORE, :], AF.Gelu_apprx_tanh)
            genc = []
            for i, (t0, tn) in enumerate(tchunks):
                et = work.tile([128, D], bf16, tag="ld", name="et")
                eng = nc.sync if i % 2 == 0 else nc.scalar
                eng.dma_start(et[:tn, :], enc_d[t0 : t0 + tn, :])
                gt = persist.tile([128, D], bf16, tag=f"genc{i}", name=f"genc{i}")
                nc.scalar.activation(gt[:tn, :], et[:tn, :], AF.Gelu_apprx_tanh)
                genc.append(gt)

            # transpose to [d, u] / [d, t]  (dec first)
            gdecT = [persist.tile([128, UCORE], bf16, tag=f"gdecT{d}", name=f"gdecT{d}") for d in range(4)]
            gencT = [persist.tile([128, T], bf16, tag=f"gencT{d}", name=f"gencT{d}") for d in range(4)]
            for dch in range(4):
                dsl = slice(dch * 128, (dch + 1) * 128)
                ps = psum.tile([128, 128], bf16, tag="tr", bufs=2)
                nc.tensor.transpose(ps[:, :UCORE], gdec[:UCORE, dsl], ident[:UCORE, :UCORE])
                nc.scalar.copy(gdecT[dch][:, :UCORE], ps[:, :UCORE])
            for dch in range(4):
                dsl = slice(dch * 128, (dch + 1) * 128)
                for i, (t0, tn) in enumerate(tchunks):
                    ps = psum.tile([128, 128], bf16, tag="tr", bufs=2)
                    nc.tensor.transpose(ps[:, :tn], genc[i][:tn, dsl], ident[:tn, :tn])
                    nc.scalar.copy(gencT[dch][:, t0 : t0 + tn], ps[:, :tn])

            # P_dec [52,640] bf16 -> DRAM bounce -> row tile at partitions 0/32/64
            pd_bf = persist.tile([3 * RG, V], bf16, tag="pd")
            nc.gpsimd.memset(pd_bf[:, :], 0.0)  # rows 52-53 stay zero (pad)
            for (v0, vn), ptag in zip(vchunks, ("pa", "pb")):
                ps = psum.tile([128, vn], f32, tag=ptag, bufs=1)
                for d in range(4):
                    nc.tensor.matmul(
                        ps[:UCORE, :vn],
                        gdecT[d][:, :UCORE],
                        w_bf[:, 4 + d, v0 : v0 + vn],
                        start=(d == 0),
                        stop=(d == 3),
                    )
                nc.scalar.copy(pd_bf[:UCORE, v0 : v0 + vn], ps[:UCORE, :vn])
            pd_dram = dramp.tile([3 * RG, V], bf16)
            nc.scalar.dma_start(pd_dram[:, :], pd_bf[:, :])
            rows = persist.tile([65, RG * V], bf16, tag="rows")
            nc.scalar.dma_start(
                rows[0:65:32, :], pd_dram[:, :].rearrange("(g x) v -> g (x v)", g=3)
            )

            # P_enc (with bias): t-chunks 0,1 packed in pe2, 44-row tail in pe3
            pe2 = persist.tile([128, 2, V], f32, tag="pe2")
            pe3 = persist.tile([128, V], f32, tag="pe3")
            for i, (t0, tn) in enumerate(tchunks):
                for (v0, vn), ptag in zip(vchunks, ("pa", "pb")):
                    ps = psum.tile([128, vn], f32, tag=ptag, bufs=1)
                    for d in range(4):
                        nc.tensor.matmul(
                            ps[:tn, :vn],
                            gencT[d][:, t0 : t0 + tn],
                            w_bf[:, d, v0 : v0 + vn],
                            start=(d == 0),
                            stop=False,
                        )
                    nc.tensor.matmul(
                        ps[:tn, :vn],
                        ones3[0:1, :tn],
                        bias_sb[:1, v0 : v0 + vn],
                        start=False,
                        stop=True,
                    )
                    dst = pe2[:, i, v0 : v0 + vn] if i < 2 else pe3[:44, v0 : v0 + vn]
                    nc.scalar.copy(dst, ps[:tn, :vn])

            # main loop: 13 blocks of 4 u
            for bi, u0 in enumerate(range(0, UCORE, UB)):
                bct4 = bctp.tile([44, UB, V], f32, tag="bct", name="bct4")
                otA = outpA.tile([128, 2, UB, V], f32, tag="otA", name="otA")
                otB = outpB.tile([44, UB, V], f32, tag="otB", name="otB")
                for j in range(UB):
                    u = u0 + j
                    g, off = u // RG, (u % RG) * V
                    ps = psum.tile([128, V], f32, tag="bc", bufs=2)
                    for c0, cn in ((0, 512), (512, V - 512)):
                        nc.tensor.matmul(
                            ps[:, c0 : c0 + cn],
                            ones3[32 * g : 32 * g + 1, :128],
                            rows[32 * g : 32 * g + 1, off + c0 : off + c0 + cn],
                            start=True,
                            stop=True,
                        )
                    nc.vector.tensor_add(
                        otA[:, :, j, :],
                        pe2[:, :, :],
                        ps[:, :].unsqueeze(1).broadcast_to([128, 2, V]),
                    )
                    nc.scalar.copy(bct4[:, j, :], ps[:44, :])
                nc.gpsimd.tensor_add(
                    otB[:, :, :],
                    pe3[:44, :].unsqueeze(1).broadcast_to([44, UB, V]),
                    bct4[:, :, :],
                )
                engA, engB = (nc.sync, nc.scalar) if bi % 2 == 0 else (nc.scalar, nc.sync)
                engA.dma_start(
                    out_d[0:256, u0 : u0 + UB, :].rearrange("(c p) j v -> p c j v", c=2),
                    otA[:, :, :, :],
                )
                engB.dma_start(out_d[256:T, u0 : u0 + UB, :], otB[:, :, :])

    nc.compile()
    return nc


def kernel(encoder_outputs, decoder_outputs, W_fc, b_fc):
    global LAST_RESULT
    import os

    import ml_dtypes
    from concourse.bass_utils import run_bass_kernel_spmd

    bf = ml_dtypes.bfloat16
    enc = np.asarray(encoder_outputs, dtype=np.float32).astype(bf)
    dec = np.asarray(decoder_outputs, dtype=np.float32)
    wT = np.ascontiguousarray(np.asarray(W_fc, dtype=np.float32).T).astype(bf)
    bias = np.asarray(b_fc, dtype=np.float32)[None, :].astype(bf)

    dec_pad = np.zeros((B, 2 * UCORE, D), dtype=np.float32)
    dec_pad[:, :U, :] = dec
    dec_pad = dec_pad.astype(bf)

    if "nc" not in _cache:
        _cache["nc"] = _build()
    nc = _cache["nc"]

    in_maps = []
    for c in range(NCORES):
        b, uh = c // 2, c % 2
        in_maps.append(
            {
                "enc": np.ascontiguousarray(enc[b]),
                "dec": np.ascontiguousarray(dec_pad[b, uh * UCORE : (uh + 1) * UCORE]),
                "wT": wT,
                "bias": bias,
            }
        )

    res = run_bass_kernel_spmd(
        nc,
        in_maps,
        list(range(NCORES)),
        trace=bool(int(os.environ.get("KJ_TRACE", "0"))),
        **RUN_KWARGS,
    )
    LAST_RESULT = res

    out = np.empty((B, T, U, V), dtype=np.float32)
    for c in range(NCORES):
        b, uh = c // 2, c % 2
        cut = res.results[c]["out"]  # (300, 52, 640)
        if uh == 0:
            out[b, :, :UCORE] = cut
        else:
            out[b, :, UCORE:U] = cut[:, : U - UCORE]
    return out


# revision 4
# speedup vs baseline: 1.7525x; 1.0216x over previous
"""RNN-T JointNet kernel for 8 Trainium2 NeuronCores.

Math: out[b,t,u,:] = gelu_tanh(concat(enc[b,t], dec[b,u])) @ W_fc^T + b_fc
Since gelu is elementwise, gelu(concat(a,b)) = concat(gelu(a), gelu(b)), so
  out[b,t,u,:] = P_enc[b,t,:] + P_dec[b,u,:]
with P_enc = gelu(enc) @ W_fc[:, :512]^T + b_fc  (small matmul, (B,T,V))
     P_dec = gelu(dec) @ W_fc[:, 512:]^T         (small matmul, (B,U,V))
The dominant cost is streaming the (B,T,U,V) = 310MB f32 output to HBM
(~111us/core at 358 GB/s); everything else must hide under the stores.

Sharding: 8 cores = 4 batches x 2 u-halves. Core c -> b = c//2, u-range
[(c%2)*52, (c%2)*52+52) of U padded 101->104. Full T=300 per core.

Inputs/weights are pre-cast to bf16 and pre-tiled on the host so every
input lands in one contiguous-per-partition DMA (10KB descriptors). P_dec
rows are staged once into a partition-{0,32,64} row tile. Per u the PE
broadcasts one P_dec row into PSUM (K=1 bf16 matmuls); one DVE op adds
pe2 (both 128-row t-chunks, stride-0 broadcast of the PSUM tile) into the
otA tile, ACT copies the 44-row tail slice to SBUF, and gpsimd adds the
t-tail for the whole 4-u block. Stores: one 2.62MB (4D AP over both
t-chunks) + one 0.45MB DMA per block, alternating the two HWDGE rings.
Prelude copies that feed PE (gencT/gdecT/pd) run on the otherwise-idle DVE
to keep the ACT FIFO short; the dec->P_dec->rows chain is issued first.
"""

import numpy as np

B, T, U = 4, 300, 101
D = 512
V = 640
UCORE = 52  # u rows per core (U padded to 104)
NCORES = 8
UB = 4  # u rows per store block (52 = 13 * 4)
RG = 18  # u rows per row-group partition (groups at partitions 0/32/64)

LAST_RESULT = None  # BassKernelResults of the most recent run (for test.py)
RUN_KWARGS = {}  # extra kwargs test.py may inject (e.g. tmpdir for traces)

_cache = {}


def _build():
    import concourse.mybir as mybir
    from concourse import bacc, masks
    from concourse.tile import TileContext

    f32 = mybir.dt.float32
    bf16 = mybir.dt.bfloat16
    AF = mybir.ActivationFunctionType

    nc = bacc.Bacc()
    # host pre-tiled: enc[p, c, :] = gelu-input row t = c*128+p (zero-padded)
    enc_d = nc.dram_tensor("enc", [128, 3, D], bf16, kind="ExternalInput")
    dec_d = nc.dram_tensor("dec", [UCORE, D], bf16, kind="ExternalInput")
    # host pre-tiled: wT[p, c, :] = W_fc.T row d = c*128+p
    wT_d = nc.dram_tensor("wT", [128, 8, V], bf16, kind="ExternalInput")
    bias_d = nc.dram_tensor("bias", [1, V], bf16, kind="ExternalInput")
    out_d = nc.dram_tensor("out", [T, UCORE, V], f32, kind="ExternalOutput")

    tchunks = [(0, 128), (128, 128), (256, 44)]
    vchunks = [(0, 512), (512, V - 512)]

    with TileContext(nc) as tc:
        with (
            tc.tile_pool(name="const", bufs=1) as constp,
            tc.tile_pool(name="work", bufs=2) as work,
            tc.tile_pool(name="persist", bufs=1) as persist,
            tc.tile_pool(name="outpA", bufs=3) as outpA,
            tc.tile_pool(name="outpB", bufs=3) as outpB,
            tc.tile_pool(name="bctp", bufs=3) as bctp,
            tc.tile_pool(name="dramp", bufs=1, space="DRAM") as dramp,
            tc.tile_pool(name="psum", bufs=1, space="PSUM") as psum,
        ):
            # input loads first: all on the sync ring, dec-path first
            dt_in = work.tile([128, D], bf16, tag="ld", name="dt_in")
            nc.sync.dma_start(dt_in[:UCORE, :], dec_d[:, :])
            et = work.tile([128, 3, D], bf16, tag="lde", name="et")
            nc.sync.dma_start(et[:, :, :], enc_d[:, :, :])
            w_bf = persist.tile([128, 8, V], bf16, tag="w")
            nc.sync.dma_start(w_bf[:, :, :], wT_d[:, :, :])
            bias_sb = constp.tile([1, V], bf16)
            nc.sync.dma_start(bias_sb[:], bias_d[:])

            ident = constp.tile([128, 128], bf16)
            masks.make_identity(nc, ident[:])
            # ones at base partitions 0/32/64 (matmul lhsT/rhs must share base)
            ones3 = constp.tile([65, 128], bf16)
            nc.gpsimd.memset(ones3[:], 1.0)

            # dummy PE op: absorbs the gpsimd-sem wait once so later
            # transposes/matmuls never carry >1 wait (S3_LW slot limit)
            warm = psum.tile([128, 128], bf16, tag="tr", bufs=2)
            nc.tensor.transpose(warm[:32, :32], ident[:32, :32], ident[:32, :32])

            # gelu: dec first (heads the deeper P_dec->rows chain)
            gdec = persist.tile([128, D], bf16, tag="gdec")
            nc.scalar.activation(gdec[:UCORE, :], dt_in[:UCORE, :], AF.Gelu_apprx_tanh)
            genc = persist.tile([128, 3, D], bf16, tag="genc")
            nc.scalar.activation(genc[:, :, :], et[:, :, :], AF.Gelu_apprx_tanh)

            # transpose to [d, u] / [d, t]; psum->SBUF copies on the idle DVE
            gdecT = [persist.tile([128, UCORE], bf16, tag=f"gdecT{d}", name=f"gdecT{d}") for d in range(4)]
            gencT = [persist.tile([128, T], bf16, tag=f"gencT{d}", name=f"gencT{d}") for d in range(4)]
            for dch in range(4):
                dsl = slice(dch * 128, (dch + 1) * 128)
                ps = psum.tile([128, 128], bf16, tag="tr", bufs=2)
                nc.tensor.transpose(ps[:, :UCORE], gdec[:UCORE, dsl], ident[:UCORE, :UCORE])
                nc.vector.tensor_copy(gdecT[dch][:, :UCORE], ps[:, :UCORE])
            for dch in range(4):
                dsl = slice(dch * 128, (dch + 1) * 128)
                for i, (t0, tn) in enumerate(tchunks):
                    ps = psum.tile([128, 128], bf16, tag="tr", bufs=2)
                    nc.tensor.transpose(ps[:, :tn], genc[:tn, i, dsl], ident[:tn, :tn])
                    nc.vector.tensor_copy(gencT[dch][:, t0 : t0 + tn], ps[:, :tn])

            # P_dec [52,640] bf16 -> DRAM bounce -> row tile at partitions 0/32/64
            pd_bf = persist.tile([3 * RG, V], bf16, tag="pd")
            nc.gpsimd.memset(pd_bf[:, :], 0.0)  # rows 52-53 stay zero (pad)
            for (v0, vn), ptag in zip(vchunks, ("pa", "pb")):
                ps = psum.tile([128, vn], f32, tag=ptag, bufs=1)
                for d in range(4):
                    nc.tensor.matmul(
                        ps[:UCORE, :vn],
                        gdecT[d][:, :UCORE],
                        w_bf[:, 4 + d, v0 : v0 + vn],
                        start=(d == 0),
                        stop=(d == 3),
                    )
                nc.vector.tensor_copy(pd_bf[:UCORE, v0 : v0 + vn], ps[:UCORE, :vn])
            pd_dram = dramp.tile([3 * RG, V], bf16)
            nc.scalar.dma_start(pd_dram[:, :], pd_bf[:, :])
            rows = persist.tile([65, RG * V], bf16, tag="rows")
            nc.scalar.dma_start(
                rows[0:65:32, :], pd_dram[:, :].rearrange("(g x) v -> g (x v)", g=3)
            )

            # P_enc (with bias): t-chunks 0,1 packed in pe2, 44-row tail in pe3
            pe2 = persist.tile([128, 2, V], f32, tag="pe2")
            pe3 = persist.tile([128, V], f32, tag="pe3")
            for i, (t0, tn) in enumerate(tchunks):
                for (v0, vn), ptag in zip(vchunks, ("pa", "pb")):
                    ps = psum.tile([128, vn], f32, tag=ptag, bufs=1)
                    for d in range(4):
                        nc.tensor.matmul(
                            ps[:tn, :vn],
                            gencT[d][:, t0 : t0 + tn],
                            w_bf[:, d, v0 : v0 + vn],
                            start=(d == 0),
                            stop=False,
                        )
                    nc.tensor.matmul(
                        ps[:tn, :vn],
                        ones3[0:1, :tn],
                        bias_sb[:1, v0 : v0 + vn],
                        start=False,
                        stop=True,
                    )
                    dst = pe2[:, i, v0 : v0 + vn] if i < 2 else pe3[:44, v0 : v0 + vn]
                    nc.scalar.copy(dst, ps[:tn, :vn])

            # main loop: 13 blocks of 4 u
            for bi, u0 in enumerate(range(0, UCORE, UB)):
                bct4 = bctp.tile([44, UB, V], f32, tag="bct", name="bct4")
                otA = outpA.tile([128, 2, UB, V], f32, tag="otA", name="otA")
                otB = outpB.tile([44, UB, V], f32, tag="otB", name="otB")
                for j in range(UB):
                    u = u0 + j
                    g, off = u // RG, (u % RG) * V
                    ps = psum.tile([128, V], f32, tag="bc", bufs=2)
                    for c0, cn in ((0, 512), (512, V - 512)):
                        nc.tensor.matmul(
                            ps[:, c0 : c0 + cn],
                            ones3[32 * g : 32 * g + 1, :128],
                            rows[32 * g : 32 * g + 1, off + c0 : off + c0 + cn],
                            start=True,
                            stop=True,
                        )
                    nc.scalar.copy(bct4[:, j, :], ps[:44, :])
                    nc.vector.tensor_add(
                        otA[:, :, j, :],
                        pe2[:, :, :],
                        ps[:, :].unsqueeze(1).broadcast_to([128, 2, V]),
                    )
                nc.gpsimd.tensor_add(
                    otB[:, :, :],
                    pe3[:44, :].unsqueeze(1).broadcast_to([44, UB, V]),
                    bct4[:, :, :],
                )
                engA, engB = (nc.sync, nc.scalar) if bi % 2 == 0 else (nc.scalar, nc.sync)
                engA.dma_start(
                    out_d[0:256, u0 : u0 + UB, :].rearrange("(c p) j v -> p c j v", c=2),
                    otA[:, :, :, :],
                )
                engB.dma_start(out_d[256:T, u0 : u0 + UB, :], otB[:, :, :])

    nc.compile()
    return nc


def kernel(encoder_outputs, decoder_outputs, W_fc, b_fc):
    global LAST_RESULT
    import os

    import ml_dtypes
    from concourse.bass_utils import run_bass_kernel_spmd

    bf = ml_dtypes.bfloat16
    enc = np.asarray(encoder_outputs, dtype=np.float32)
    dec = np.asarray(decoder_outputs, dtype=np.float32)

    # enc per batch -> [128, 3, 512] with row t = c*128 + p, zero padded
    enc_pad = np.zeros((B, 384, D), dtype=np.float32)
    enc_pad[:, :T, :] = enc
    enc_tiled = np.ascontiguousarray(
        enc_pad.reshape(B, 3, 128, D).transpose(0, 2, 1, 3)
    ).astype(bf)

    # W_fc.T -> [128, 8, 640] with row d = c*128 + p
    wT = np.asarray(W_fc, dtype=np.float32).T  # (1024, 640)
    wT_tiled = np.ascontiguousarray(
        wT.reshape(8, 128, V).transpose(1, 0, 2)
    ).astype(bf)

    bias = np.asarray(b_fc, dtype=np.float32)[None, :].astype(bf)

    dec_pad = np.zeros((B, 2 * UCORE, D), dtype=np.float32)
    dec_pad[:, :U, :] = dec
    dec_pad = dec_pad.astype(bf)

    if "nc" not in _cache:
        _cache["nc"] = _build()
    nc = _cache["nc"]

    in_maps = []
    for c in range(NCORES):
        b, uh = c // 2, c % 2
        in_maps.append(
            {
                "enc": enc_tiled[b],
                "dec": np.ascontiguousarray(dec_pad[b, uh * UCORE : (uh + 1) * UCORE]),
                "wT": wT_tiled,
                "bias": bias,
            }
        )

    res = run_bass_kernel_spmd(
        nc,
        in_maps,
        list(range(NCORES)),
        trace=bool(int(os.environ.get("KJ_TRACE", "0"))),
        **RUN_KWARGS,
    )
    LAST_RESULT = res

    out = np.empty((B, T, U, V), dtype=np.float32)
    for c in range(NCORES):
        b, uh = c // 2, c % 2
        cut = res.results[c]["out"]  # (300, 52, 640)
        if uh == 0:
            out[b, :, :UCORE] = cut
        else:
            out[b, :, UCORE:U] = cut[:, : U - UCORE]
    return out


# revision 5
# speedup vs baseline: 1.7919x; 1.0225x over previous
"""RNN-T JointNet kernel for 8 Trainium2 NeuronCores.

Math: out[b,t,u,:] = gelu_tanh(concat(enc[b,t], dec[b,u])) @ W_fc^T + b_fc
Since gelu is elementwise, gelu(concat(a,b)) = concat(gelu(a), gelu(b)), so
  out[b,t,u,:] = P_enc[b,t,:] + P_dec[b,u,:]
with P_enc = gelu(enc) @ W_fc[:, :512]^T + b_fc  (small matmul, (B,T,V))
     P_dec = gelu(dec) @ W_fc[:, 512:]^T         (small matmul, (B,U,V))
The dominant cost is streaming the (B,T,U,V) = 310MB f32 output to HBM
(~111us/core at 358 GB/s); everything else must hide under the stores.

Sharding: 8 cores = 4 batches x 2 u-halves. Core c -> b = c//2, u-range
[(c%2)*52, (c%2)*52+52) of U padded 101->104. Full T=300 per core.

Inputs/weights are pre-cast to bf16 and pre-tiled on the host so every
input lands in one contiguous-per-partition DMA; the weight DMA is
dispatched first since it gates the matmul phase. A burst of dummy
transposes keeps the PE HAM clock-gate open (2.4GHz) before the real
matmuls. Each P_enc/P_dec matmul group accumulates both v-chunks into one
[128,640] PSUM tile (shared tag with the loop broadcasts; 3 bufs + 2
transpose banks = 8 PSUM banks). P_dec rows are relayouted to a
partition-{0,32,64} row tile with one SBUF->SBUF DMA (no DRAM bounce).
Per u the PE broadcasts one P_dec row into PSUM (K=1 bf16 matmuls); one
DVE op adds pe2 (both 128-row t-chunks, stride-0 broadcast of the PSUM
tile) into the otA tile, ACT copies the 44-row tail slice to SBUF, and
gpsimd adds the t-tail for the whole 4-u block. Stores: one 2.62MB (4D AP
over both t-chunks) + one 0.45MB DMA per block, alternating HWDGE rings.
"""

import numpy as np

B, T, U = 4, 300, 101
D = 512
V = 640
UCORE = 52  # u rows per core (U padded to 104)
NCORES = 8
UB = 4  # u rows per store block (52 = 13 * 4)
RG = 18  # u rows per row-group partition (groups at partitions 0/32/64)
NWARM = 75  # dummy PE transposes to hold the HAM clock-gate open

LAST_RESULT = None  # BassKernelResults of the most recent run (for test.py)
RUN_KWARGS = {}  # extra kwargs test.py may inject (e.g. tmpdir for traces)

_cache = {}


def _build():
    import concourse.mybir as mybir
    from concourse import bacc, masks
    from concourse.tile import TileContext

    f32 = mybir.dt.float32
    bf16 = mybir.dt.bfloat16
    AF = mybir.ActivationFunctionType

    nc = bacc.Bacc()
    # host pre-tiled: enc[p, c, :] = gelu-input row t = c*128+p (zero-padded)
    enc_d = nc.dram_tensor("enc", [128, 3, D], bf16, kind="ExternalInput")
    dec_d = nc.dram_tensor("dec", [UCORE, D], bf16, kind="ExternalInput")
    # host pre-tiled: wT[p, c, :] = W_fc.T row d = c*128+p
    wT_d = nc.dram_tensor("wT", [128, 8, V], bf16, kind="ExternalInput")
    bias_d = nc.dram_tensor("bias", [1, V], bf16, kind="ExternalInput")
    out_d = nc.dram_tensor("out", [T, UCORE, V], f32, kind="ExternalOutput")

    tchunks = [(0, 128), (128, 128), (256, 44)]
    vchunks = [(0, 512), (512, V - 512)]

    with TileContext(nc) as tc:
        with (
            tc.tile_pool(name="const", bufs=1) as constp,
            tc.tile_pool(name="work", bufs=2) as work,
            tc.tile_pool(name="persist", bufs=1) as persist,
            tc.tile_pool(name="outpA", bufs=3) as outpA,
            tc.tile_pool(name="outpB", bufs=3) as outpB,
            tc.tile_pool(name="bctp", bufs=3) as bctp,
            tc.tile_pool(name="psum", bufs=1, space="PSUM") as psum,
        ):
            # input loads: all on the sync ring; w first (it gates matmuls)
            w_bf = persist.tile([128, 8, V], bf16, tag="w")
            nc.sync.dma_start(w_bf[:, :, :], wT_d[:, :, :])
            dt_in = work.tile([128, D], bf16, tag="ld", name="dt_in")
            nc.sync.dma_start(dt_in[:UCORE, :], dec_d[:, :])
            et = work.tile([128, 3, D], bf16, tag="lde", name="et")
            nc.sync.dma_start(et[:, :, :], enc_d[:, :, :])
            bias_sb = constp.tile([1, V], bf16)
            nc.sync.dma_start(bias_sb[:], bias_d[:])

            ident = constp.tile([128, 128], bf16)
            masks.make_identity(nc, ident[:])
            # ones at base partitions 0/32/64 (matmul lhsT/rhs must share base)
            ones3 = constp.tile([65, 128], bf16)
            nc.gpsimd.memset(ones3[:], 1.0)

            # dummy PE ops: absorb the gpsimd-sem wait AND keep the PE HAM
            # activity window busy until real matmuls arrive, so they run at
            # 2.4GHz instead of the cold 1.2GHz
            warm = psum.tile([128, 128], bf16, tag="tr", bufs=2)
            for _ in range(NWARM):
                nc.tensor.transpose(warm[:, :], ident[:, :], ident[:, :])

            # gelu: dec first (heads the deeper P_dec->rows chain)
            gdec = persist.tile([128, D], bf16, tag="gdec")
            nc.scalar.activation(gdec[:UCORE, :], dt_in[:UCORE, :], AF.Gelu_apprx_tanh)
            genc = persist.tile([128, 3, D], bf16, tag="genc")
            nc.scalar.activation(genc[:, :, :], et[:, :, :], AF.Gelu_apprx_tanh)

            # transpose to [d, u] / [d, t]; psum->SBUF copies on the idle DVE
            gdecT = [persist.tile([128, UCORE], bf16, tag=f"gdecT{d}", name=f"gdecT{d}") for d in range(4)]
            gencT = [persist.tile([128, 384], bf16, tag=f"gencT{d}", name=f"gencT{d}") for d in range(4)]
            for dch in range(4):
                dsl = slice(dch * 128, (dch + 1) * 128)
                ps = psum.tile([128, 128], bf16, tag="tr", bufs=2)
                nc.tensor.transpose(ps[:, :UCORE], gdec[:UCORE, dsl], ident[:UCORE, :UCORE])
                nc.vector.tensor_copy(gdecT[dch][:, :UCORE], ps[:, :UCORE])
            for dch in range(4):
                dsl = slice(dch * 128, (dch + 1) * 128)
                for i in range(3):
                    ps = psum.tile([128, 128], bf16, tag="tr", bufs=2)
                    nc.tensor.transpose(ps[:, :], genc[:, i, dsl], ident[:, :])
                    nc.vector.tensor_copy(gencT[dch][:, i * 128 : (i + 1) * 128], ps[:, :])

            # P_dec [52,640] bf16 -> SBUF->SBUF DMA relayout to row tile at
            # partitions 0/32/64
            pd_bf = persist.tile([3 * RG, V], bf16, tag="pd")
            nc.gpsimd.memset(pd_bf[:, :], 0.0)  # rows 52-53 stay zero (pad)
            ps = psum.tile([128, V], f32, tag="bc", bufs=3)
            for v0, vn in vchunks:
                for d in range(4):
                    nc.tensor.matmul(
                        ps[:UCORE, v0 : v0 + vn],
                        gdecT[d][:, :UCORE],
                        w_bf[:, 4 + d, v0 : v0 + vn],
                        start=(d == 0),
                        stop=(d == 3),
                    )
            nc.vector.tensor_copy(pd_bf[:UCORE, :], ps[:UCORE, :])
            rows = persist.tile([65, RG * V], bf16, tag="rows")
            nc.scalar.dma_start(rows[0:65:32, :], pd_bf[:, :])

            # P_enc (with bias): t-chunks 0,1 packed in pe2, 44-row tail in pe3
            pe2 = persist.tile([128, 2, V], f32, tag="pe2")
            pe3 = persist.tile([128, V], f32, tag="pe3")
            for i, (t0, tn) in enumerate(tchunks):
                ps = psum.tile([128, V], f32, tag="bc", bufs=3)
                for v0, vn in vchunks:
                    for d in range(4):
                        nc.tensor.matmul(
                            ps[:tn, v0 : v0 + vn],
                            gencT[d][:, t0 : t0 + tn],
                            w_bf[:, d, v0 : v0 + vn],
                            start=(d == 0),
                            stop=False,
                        )
                    nc.tensor.matmul(
                        ps[:tn, v0 : v0 + vn],
                        ones3[0:1, :tn],
                        bias_sb[:1, v0 : v0 + vn],
                        start=False,
                        stop=True,
                    )
                dst = pe2[:, i, :] if i < 2 else pe3[:44, :]
                nc.scalar.copy(dst, ps[:tn, :])

            # main loop: 13 blocks of 4 u
            for bi, u0 in enumerate(range(0, UCORE, UB)):
                bct4 = bctp.tile([44, UB, V], f32, tag="bct", name="bct4")
                otA = outpA.tile([128, 2, UB, V], f32, tag="otA", name="otA")
                otB = outpB.tile([44, UB, V], f32, tag="otB", name="otB")
                for j in range(UB):
                    u = u0 + j
                    g, off = u // RG, (u % RG) * V
                    ps = psum.tile([128, V], f32, tag="bc", bufs=3)
                    for c0, cn in ((0, 512), (512, V - 512)):
                        nc.tensor.matmul(
                            ps[:, c0 : c0 + cn],
                            ones3[32 * g : 32 * g + 1, :128],
                            rows[32 * g : 32 * g + 1, off + c0 : off + c0 + cn],
                            start=True,
                            stop=True,
                        )
                    nc.scalar.copy(bct4[:, j, :], ps[:44, :])
                    nc.vector.tensor_add(
                        otA[:, :, j, :],
                        pe2[:, :, :],
                        ps[:, :].unsqueeze(1).broadcast_to([128, 2, V]),
                    )
                nc.gpsimd.tensor_add(
                    otB[:, :, :],
                    pe3[:44, :].unsqueeze(1).broadcast_to([44, UB, V]),
                    bct4[:, :, :],
                )
                engA, engB = (nc.sync, nc.scalar) if bi % 2 == 0 else (nc.scalar, nc.sync)
                engA.dma_start(
                    out_d[0:256, u0 : u0 + UB, :].rearrange("(c p) j v -> p c j v", c=2),
                    otA[:, :, :, :],
                )
                engB.dma_start(out_d[256:T, u0 : u0 + UB, :], otB[:, :, :])

    nc.compile()
    return nc


def kernel(encoder_outputs, decoder_outputs, W_fc, b_fc):
    global LAST_RESULT
    import os

    import ml_dtypes
    from concourse.bass_utils import run_bass_kernel_spmd

    bf = ml_dtypes.bfloat16
    enc = np.asarray(encoder_outputs, dtype=np.float32)
    dec = np.asarray(decoder_outputs, dtype=np.float32)

    # enc per batch -> [128, 3, 512] with row t = c*128 + p, zero padded
    enc_pad = np.zeros((B, 384, D), dtype=np.float32)
    enc_pad[:, :T, :] = enc
    enc_tiled = np.ascontiguousarray(
        enc_pad.reshape(B, 3, 128, D).transpose(0, 2, 1, 3)
    ).astype(bf)

    # W_fc.T -> [128, 8, 640] with row d = c*128 + p
    wT = np.asarray(W_fc, dtype=np.float32).T  # (1024, 640)
    wT_tiled = np.ascontiguousarray(
        wT.reshape(8, 128, V).transpose(1, 0, 2)
    ).astype(bf)

    bias = np.asarray(b_fc, dtype=np.float32)[None, :].astype(bf)

    dec_pad = np.zeros((B, 2 * UCORE, D), dtype=np.float32)
    dec_pad[:, :U, :] = dec
    dec_pad = dec_pad.astype(bf)

    if "nc" not in _cache:
        _cache["nc"] = _build()
    nc = _cache["nc"]

    in_maps = []
    for c in range(NCORES):
        b, uh = c // 2, c % 2
        in_maps.append(
            {
                "enc": enc_tiled[b],
                "dec": np.ascontiguousarray(dec_pad[b, uh * UCORE : (uh + 1) * UCORE]),
                "wT": wT_tiled,
                "bias": bias,
            }
        )

    res = run_bass_kernel_spmd(
        nc,
        in_maps,
        list(range(NCORES)),
        trace=bool(int(os.environ.get("KJ_TRACE", "0"))),
        **RUN_KWARGS,
    )
    LAST_RESULT = res

    out = np.empty((B, T, U, V), dtype=np.float32)
    for c in range(NCORES):
        b, uh = c // 2, c % 2
        cut = res.results[c]["out"]  # (300, 52, 640)
        if uh == 0:
            out[b, :, :UCORE] = cut
        else:
            out[b, :, UCORE:U] = cut[:, : U - UCORE]
    return out


# revision 9
# speedup vs baseline: 1.7933x; 1.0008x over previous
"""RNN-T JointNet kernel for 8 Trainium2 NeuronCores.

Math: out[b,t,u,:] = gelu_tanh(concat(enc[b,t], dec[b,u])) @ W_fc^T + b_fc
Since gelu is elementwise, gelu(concat(a,b)) = concat(gelu(a), gelu(b)), so
  out[b,t,u,:] = P_enc[b,t,:] + P_dec[b,u,:]
with P_enc = gelu(enc) @ W_fc[:, :512]^T + b_fc  (small matmul, (B,T,V))
     P_dec = gelu(dec) @ W_fc[:, 512:]^T         (small matmul, (B,U,V))
The dominant cost is streaming the (B,T,U,V) = 310MB f32 output to HBM
(~111us/core at 358 GB/s); everything else must hide under the stores.

Sharding: 8 cores = 4 batches x 2 u-halves. Core c -> b = c//2, u-range
[(c%2)*52, (c%2)*52+52) of U padded 101->104. Full T=300 per core.

Inputs/weights are pre-cast to bf16 and pre-tiled on the host so every
input lands in one contiguous-per-partition DMA; the weight DMA is
dispatched first since it gates the matmul phase. A burst of dummy
transposes keeps the PE HAM clock-gate open (2.4GHz) before the real
matmuls. Each P_enc/P_dec matmul group accumulates both v-chunks into one
[128,640] PSUM tile (shared tag with the loop broadcasts; 3 bufs + 2
transpose banks = 8 PSUM banks). P_dec rows are relayouted to a
partition-{0,32,64} row tile with one SBUF->SBUF DMA (no DRAM bounce).
Per u the PE broadcasts one P_dec row into PSUM (K=1 bf16 matmuls); one
DVE op adds pe2 (both 128-row t-chunks, stride-0 broadcast of the PSUM
tile) into the otA tile, ACT copies the 44-row tail slice to SBUF, and
gpsimd adds the t-tail for the whole 4-u block. Stores: one 2.62MB (4D AP
over both t-chunks) + one 0.45MB DMA per block, alternating HWDGE rings.
"""

import numpy as np

B, T, U = 4, 300, 101
D = 512
V = 640
UCORE = 52  # u rows per core (U padded to 104)
NCORES = 8
UB = 4  # u rows per store block (52 = 13 * 4)
RG = 18  # u rows per row-group partition (groups at partitions 0/32/64)
NWARM = 75  # dummy PE transposes to hold the HAM clock-gate open

LAST_RESULT = None  # BassKernelResults of the most recent run (for test.py)
RUN_KWARGS = {}  # extra kwargs test.py may inject (e.g. tmpdir for traces)

_cache = {}


def _build():
    import concourse.mybir as mybir
    from concourse import bacc, masks
    from concourse.tile import TileContext

    f32 = mybir.dt.float32
    bf16 = mybir.dt.bfloat16
    AF = mybir.ActivationFunctionType

    nc = bacc.Bacc()
    # host pre-tiled: enc[p, c, :] = gelu-input row t = c*128+p (zero-padded)
    enc_d = nc.dram_tensor("enc", [128, 3, D], bf16, kind="ExternalInput")
    dec_d = nc.dram_tensor("dec", [UCORE, D], bf16, kind="ExternalInput")
    # host pre-tiled: wT[p, c, :] = W_fc.T row d = c*128+p
    wT_d = nc.dram_tensor("wT", [128, 8, V], bf16, kind="ExternalInput")
    bias_d = nc.dram_tensor("bias", [1, V], bf16, kind="ExternalInput")
    # outputs laid out exactly like the SBUF tiles so every store is one
    # fully contiguous DRAM write (best HBM locality); host un-permutes.
    # outA[bi, p, c, j, v] = out[t = c*128 + p, u = 4*bi + j, v]
    # outB[bi, p, j, v]    = out[t = 256 + p,   u = 4*bi + j, v]
    NBLK = UCORE // UB
    outA_d = nc.dram_tensor("outA", [NBLK, 128, 2, UB, V], f32, kind="ExternalOutput")
    outB_d = nc.dram_tensor("outB", [NBLK, 44, UB, V], f32, kind="ExternalOutput")

    tchunks = [(0, 128), (128, 128), (256, 44)]
    vchunks = [(0, 512), (512, V - 512)]

    with TileContext(nc) as tc:
        with (
            tc.tile_pool(name="const", bufs=1) as constp,
            tc.tile_pool(name="work", bufs=2) as work,
            tc.tile_pool(name="persist", bufs=1) as persist,
            tc.tile_pool(name="outpA", bufs=3) as outpA,
            tc.tile_pool(name="outpB", bufs=3) as outpB,
            tc.tile_pool(name="bctp", bufs=3) as bctp,
            tc.tile_pool(name="psum", bufs=1, space="PSUM") as psum,
        ):
            # input loads: small gelu inputs first (they head the compute
            # chains), split across both HWDGE rings; w queued right behind
            dt_in = work.tile([128, D], bf16, tag="ld", name="dt_in")
            nc.sync.dma_start(dt_in[:UCORE, :], dec_d[:, :])
            et = work.tile([128, 3, D], bf16, tag="lde", name="et")
            nc.scalar.dma_start(et[:, :, :], enc_d[:, :, :])
            w_bf = persist.tile([128, 8, V], bf16, tag="w")
            nc.sync.dma_start(w_bf[:, :, :], wT_d[:, :, :])
            bias_sb = constp.tile([1, V], bf16)
            nc.scalar.dma_start(bias_sb[:], bias_d[:])

            ident = constp.tile([128, 128], bf16)
            masks.make_identity(nc, ident[:])
            # ones at base partitions 0/32/64 (matmul lhsT/rhs must share base)
            ones3 = constp.tile([65, 128], bf16)
            nc.gpsimd.memset(ones3[:], 1.0)

            # dummy PE ops: absorb the gpsimd-sem wait AND keep the PE HAM
            # activity window busy until real matmuls arrive, so they run at
            # 2.4GHz instead of the cold 1.2GHz
            warm = psum.tile([128, 128], bf16, tag="tr", bufs=2)
            for _ in range(NWARM):
                nc.tensor.transpose(warm[:, :], ident[:, :], ident[:, :])

            # gelu: dec first (heads the deeper P_dec->rows chain)
            gdec = persist.tile([128, D], bf16, tag="gdec")
            nc.scalar.activation(gdec[:UCORE, :], dt_in[:UCORE, :], AF.Gelu_apprx_tanh)
            genc = persist.tile([128, 3, D], bf16, tag="genc")
            nc.scalar.activation(genc[:, :, :], et[:, :, :], AF.Gelu_apprx_tanh)

            # transpose to [d, u] / [d, t]; psum->SBUF copies on the idle DVE
            gdecT = [persist.tile([128, UCORE], bf16, tag=f"gdecT{d}", name=f"gdecT{d}") for d in range(4)]
            gencT = [persist.tile([128, 384], bf16, tag=f"gencT{d}", name=f"gencT{d}") for d in range(4)]
            for dch in range(4):
                dsl = slice(dch * 128, (dch + 1) * 128)
                ps = psum.tile([128, 128], bf16, tag="tr", bufs=2)
                nc.tensor.transpose(ps[:, :UCORE], gdec[:UCORE, dsl], ident[:UCORE, :UCORE])
                nc.vector.tensor_copy(gdecT[dch][:, :UCORE], ps[:, :UCORE])
            for dch in range(4):
                dsl = slice(dch * 128, (dch + 1) * 128)
                for i in range(3):
                    ps = psum.tile([128, 128], bf16, tag="tr", bufs=2)
                    nc.tensor.transpose(ps[:, :], genc[:, i, dsl], ident[:, :])
                    nc.vector.tensor_copy(gencT[dch][:, i * 128 : (i + 1) * 128], ps[:, :])

            # P_dec [52,640] bf16 -> SBUF->SBUF DMA relayout to row tile at
            # partitions 0/32/64
            pd_bf = persist.tile([3 * RG, V], bf16, tag="pd")
            nc.gpsimd.memset(pd_bf[:, :], 0.0)  # rows 52-53 stay zero (pad)
            ps = psum.tile([128, V], f32, tag="bc", bufs=3)
            for v0, vn in vchunks:
                for d in range(4):
                    nc.tensor.matmul(
                        ps[:UCORE, v0 : v0 + vn],
                        gdecT[d][:, :UCORE],
                        w_bf[:, 4 + d, v0 : v0 + vn],
                        start=(d == 0),
                        stop=(d == 3),
                    )
            nc.vector.tensor_copy(pd_bf[:UCORE, :], ps[:UCORE, :])
            rows = persist.tile([65, RG * V], bf16, tag="rows")
            nc.scalar.dma_start(rows[0:65:32, :], pd_bf[:, :])

            # P_enc (with bias): t-chunks 0,1 packed in pe2, 44-row tail in pe3
            pe2 = persist.tile([128, 2, V], f32, tag="pe2")
            pe3 = persist.tile([128, V], f32, tag="pe3")
            for i, (t0, tn) in enumerate(tchunks):
                ps = psum.tile([128, V], f32, tag="bc", bufs=3)
                for v0, vn in vchunks:
                    for d in range(4):
                        nc.tensor.matmul(
                            ps[:tn, v0 : v0 + vn],
                            gencT[d][:, t0 : t0 + tn],
                            w_bf[:, d, v0 : v0 + vn],
                            start=(d == 0),
                            stop=False,
                        )
                    nc.tensor.matmul(
                        ps[:tn, v0 : v0 + vn],
                        ones3[0:1, :tn],
                        bias_sb[:1, v0 : v0 + vn],
                        start=False,
                        stop=True,
                    )
                dst = pe2[:, i, :] if i < 2 else pe3[:44, :]
                nc.scalar.copy(dst, ps[:tn, :])

            # main loop: 13 blocks of 4 u
            for bi, u0 in enumerate(range(0, UCORE, UB)):
                bct4 = bctp.tile([44, UB, V], f32, tag="bct", name="bct4")
                otA = outpA.tile([128, 2, UB, V], f32, tag="otA", name="otA")
                otB = outpB.tile([44, UB, V], f32, tag="otB", name="otB")
                for j in range(UB):
                    u = u0 + j
                    g, off = u // RG, (u % RG) * V
                    ps = psum.tile([128, V], f32, tag="bc", bufs=3)
                    for c0, cn in ((0, 512), (512, V - 512)):
                        nc.tensor.matmul(
                            ps[:, c0 : c0 + cn],
                            ones3[32 * g : 32 * g + 1, :128],
                            rows[32 * g : 32 * g + 1, off + c0 : off + c0 + cn],
                            start=True,
                            stop=True,
                        )
                    nc.scalar.copy(bct4[:, j, :], ps[:44, :])
                    nc.vector.tensor_add(
                        otA[:, :, j, :],
                        pe2[:, :, :],
                        ps[:, :].unsqueeze(1).broadcast_to([128, 2, V]),
                    )
                nc.gpsimd.tensor_add(
                    otB[:, :, :],
                    pe3[:44, :].unsqueeze(1).broadcast_to([44, UB, V]),
                    bct4[:, :, :],
                )
                engA, engB = (nc.sync, nc.scalar) if bi % 2 == 0 else (nc.scalar, nc.sync)
                if bi < UCORE // UB - 1:
                    engA.dma_start(outA_d[bi, :, :, :, :], otA[:, :, :, :])
                    engB.dma_start(outB_d[bi, :, :, :], otB[:, :, :])
                else:
                    # last block: split the big store across both rings so the
                    # drain is half as long
                    engA.dma_start(outA_d[bi, :, 0, :, :], otA[:, 0, :, :])
                    engB.dma_start(outA_d[bi, :, 1, :, :], otA[:, 1, :, :])
                    engA.dma_start(outB_d[bi, :, :, :], otB[:, :, :])

    nc.compile()
    return nc


def kernel(encoder_outputs, decoder_outputs, W_fc, b_fc):
    global LAST_RESULT
    import os

    import ml_dtypes
    from concourse.bass_utils import run_bass_kernel_spmd

    bf = ml_dtypes.bfloat16
    enc = np.asarray(encoder_outputs, dtype=np.float32)
    dec = np.asarray(decoder_outputs, dtype=np.float32)

    # enc per batch -> [128, 3, 512] with row t = c*128 + p, zero padded
    enc_pad = np.zeros((B, 384, D), dtype=np.float32)
    enc_pad[:, :T, :] = enc
    enc_tiled = np.ascontiguousarray(
        enc_pad.reshape(B, 3, 128, D).transpose(0, 2, 1, 3)
    ).astype(bf)

    # W_fc.T -> [128, 8, 640] with row d = c*128 + p
    wT = np.asarray(W_fc, dtype=np.float32).T  # (1024, 640)
    wT_tiled = np.ascontiguousarray(
        wT.reshape(8, 128, V).transpose(1, 0, 2)
    ).astype(bf)

    bias = np.asarray(b_fc, dtype=np.float32)[None, :].astype(bf)

    dec_pad = np.zeros((B, 2 * UCORE, D), dtype=np.float32)
    dec_pad[:, :U, :] = dec
    dec_pad = dec_pad.astype(bf)

    if "nc" not in _cache:
        _cache["nc"] = _build()
    nc = _cache["nc"]

    in_maps = []
    for c in range(NCORES):
        b, uh = c // 2, c % 2
        in_maps.append(
            {
                "enc": enc_tiled[b],
                "dec": np.ascontiguousarray(dec_pad[b, uh * UCORE : (uh + 1) * UCORE]),
                "wT": wT_tiled,
                "bias": bias,
            }
        )

    res = run_bass_kernel_spmd(
        nc,
        in_maps,
        list(range(NCORES)),
        trace=bool(int(os.environ.get("KJ_TRACE", "0"))),
        **RUN_KWARGS,
    )
    LAST_RESULT = res

    out = np.empty((B, T, U, V), dtype=np.float32)
    for c in range(NCORES):
        b, uh = c // 2, c % 2
        # outA (13,128,2,4,640): [bi,p,cc,j,v] -> t = cc*128+p, u = 4*bi+j
        # outB (13,44,4,640):    [bi,p,j,v]    -> t = 256+p,    u = 4*bi+j
        outA = res.results[c]["outA"]
        outB = res.results[c]["outB"]
        cut = np.empty((T, UCORE, V), dtype=np.float32)
        cut[:256] = outA.transpose(2, 1, 0, 3, 4).reshape(256, UCORE, V)
        cut[256:] = outB.transpose(1, 0, 2, 3).reshape(44, UCORE, V)
        if uh == 0:
            out[b, :, :UCORE] = cut
        else:
            out[b, :, UCORE:U] = cut[:, : U - UCORE]
    return out


# revision 32
# speedup vs baseline: 1.9028x; 1.0611x over previous
"""RNN-T JointNet kernel for 8 Trainium2 NeuronCores.

Math: out[b,t,u,:] = gelu_tanh(concat(enc[b,t], dec[b,u])) @ W_fc^T + b_fc
Since gelu is elementwise, gelu(concat(a,b)) = concat(gelu(a), gelu(b)), so
  out[b,t,u,:] = P_enc[b,t,:] + P_dec[b,u,:]
with P_enc = gelu(enc) @ W_fc[:, :512]^T + b_fc  (small matmul, (B,T,V))
     P_dec = gelu(dec) @ W_fc[:, 512:]^T         (small matmul, (B,U,V))
The dominant cost is streaming the (B,T,U,V) = 310MB f32 output to HBM
(~111us/core at 358 GB/s); everything else must hide under the stores.

Sharding: 8 cores = 4 batches x 2 u-halves. Core c -> b = c//2, u-range
[(c%2)*52, (c%2)*52+52) of U padded 101->104. Full T=300 per core.

Inputs/weights are pre-cast to bf16 and pre-tiled on the host so every
input lands in one contiguous-per-partition DMA; the weight DMA is
dispatched first since it gates the matmul phase. A burst of dummy
transposes keeps the PE HAM clock-gate open (2.4GHz) before the real
matmuls. Each P_enc/P_dec matmul group accumulates both v-chunks into one
[128,640] PSUM tile (shared tag with the loop broadcasts; 3 bufs + 2
transpose banks = 8 PSUM banks). P_dec rows are relayouted to a
partition-{0,32,64} row tile with one SBUF->SBUF DMA (no DRAM bounce).
Per u the PE broadcasts one P_dec row into PSUM (K=1 bf16 matmuls); one
DVE op adds pe2 (both 128-row t-chunks, stride-0 broadcast of the PSUM
tile) into the otA tile, ACT copies the 44-row tail slice to SBUF, and
gpsimd adds the t-tail for the whole 4-u block. Stores: one 2.62MB (4D AP
over both t-chunks) + one 0.45MB DMA per block, alternating HWDGE rings.
"""

import numpy as np

B, T, U = 4, 300, 101
D = 512
V = 640
UCORE = 52  # u rows per core (U padded to 104)
NCORES = 8
UB = 4  # u rows per store block (52 = 13 * 4)
RG = 18  # u rows per row-group partition (groups at partitions 0/32/64)
NWARM = 45  # dummy PE transposes to hold the HAM clock-gate open

LAST_RESULT = None  # BassKernelResults of the most recent run (for test.py)
RUN_KWARGS = {}  # extra kwargs test.py may inject (e.g. tmpdir for traces)

_cache = {}


def _build():
    import concourse.mybir as mybir
    from concourse import bacc, masks
    from concourse.tile import TileContext

    f32 = mybir.dt.float32
    bf16 = mybir.dt.bfloat16
    AF = mybir.ActivationFunctionType

    nc = bacc.Bacc()
    # host pre-tiled: enc[p, c, :] = gelu-input row t = c*128+p (zero-padded)
    enc_d = nc.dram_tensor("enc", [128, 3, D], bf16, kind="ExternalInput")
    dec_d = nc.dram_tensor("dec", [UCORE, D], bf16, kind="ExternalInput")
    # host pre-tiled: wT[p, c, :] = W_fc.T row d = c*128+p
    wT_d = nc.dram_tensor("wT", [128, 8, V], bf16, kind="ExternalInput")
    bias_d = nc.dram_tensor("bias", [1, V], bf16, kind="ExternalInput")
    # outputs laid out exactly like the SBUF tiles so every store is one
    # fully contiguous DRAM write (best HBM locality); host un-permutes.
    # outA[bi, p, c, j, v] = out[t = c*128 + p, u = 4*bi + j, v]
    # outB[bi, p, j, v]    = out[t = 256 + p,   u = 4*bi + j, v]
    NBLK = UCORE // UB
    outA_d = nc.dram_tensor("outA", [NBLK, 128, 2, UB, V], f32, kind="ExternalOutput")
    outB_d = nc.dram_tensor("outB", [NBLK, 44, UB, V], f32, kind="ExternalOutput")

    tchunks = [(0, 128), (128, 128), (256, 44)]
    vchunks = [(0, 512), (512, V - 512)]

    with TileContext(nc) as tc:
        with (
            tc.tile_pool(name="const", bufs=1) as constp,
            tc.tile_pool(name="work", bufs=2) as work,
            tc.tile_pool(name="persist", bufs=1) as persist,
            tc.tile_pool(name="outpA", bufs=3) as outpA,
            tc.tile_pool(name="outpB", bufs=3) as outpB,
            tc.tile_pool(name="bctp", bufs=3) as bctp,
            tc.tile_pool(name="psum", bufs=1, space="PSUM") as psum,
        ):
            # input loads: small gelu inputs first (they head the compute
            # chains), split across both HWDGE rings; w queued right behind
            dt_in = work.tile([128, D], bf16, tag="ld", name="dt_in")
            nc.sync.dma_start(dt_in[:UCORE, :], dec_d[:, :])
            et = work.tile([128, 3, D], bf16, tag="lde", name="et")
            nc.scalar.dma_start(et[:, :, :], enc_d[:, :, :])
            # w on the SWDGE (gpsimd) ring: dispatches early (no ACT table
            # loads ahead of it) and overlaps the HWDGE input loads
            w_bf = persist.tile([128, 8, V], bf16, tag="w")
            nc.gpsimd.dma_start(w_bf[:, :, :], wT_d[:, :, :])
            bias_sb = constp.tile([1, V], bf16)
            nc.scalar.dma_start(bias_sb[:], bias_d[:])

            ident = constp.tile([128, 128], bf16)
            masks.make_identity(nc, ident[:])
            # ones at base partitions 0/32/64 (matmul lhsT/rhs must share base)
            ones3 = constp.tile([65, 128], bf16)
            nc.gpsimd.memset(ones3[:], 1.0)

            # dummy PE ops: absorb the gpsimd-sem wait AND keep the PE HAM
            # activity window busy until real matmuls arrive, so they run at
            # 2.4GHz instead of the cold 1.2GHz
            warm = psum.tile([128, 128], bf16, tag="tr", bufs=2)
            for _ in range(NWARM):
                nc.tensor.transpose(warm[:, :], ident[:, :], ident[:, :])

            # gelu: dec first (heads the deeper P_dec->rows chain)
            gdec = persist.tile([128, D], bf16, tag="gdec")
            nc.scalar.activation(gdec[:UCORE, :], dt_in[:UCORE, :], AF.Gelu_apprx_tanh)
            genc = persist.tile([128, 3, D], bf16, tag="genc")
            nc.scalar.activation(genc[:, :, :], et[:, :, :], AF.Gelu_apprx_tanh)

            # transpose to [d, u] / [d, t]; psum->SBUF copies on the idle DVE
            gdecT = [persist.tile([128, UCORE], bf16, tag=f"gdecT{d}", name=f"gdecT{d}") for d in range(4)]
            gencT = [persist.tile([128, 384], bf16, tag=f"gencT{d}", name=f"gencT{d}") for d in range(4)]
            for dch in range(4):
                dsl = slice(dch * 128, (dch + 1) * 128)
                ps = psum.tile([128, 128], bf16, tag="tr", bufs=2)
                nc.tensor.transpose(ps[:, :UCORE], gdec[:UCORE, dsl], ident[:UCORE, :UCORE])
                nc.vector.tensor_copy(gdecT[dch][:, :UCORE], ps[:, :UCORE])
            for dch in range(4):
                dsl = slice(dch * 128, (dch + 1) * 128)
                for i in range(3):
                    ps = psum.tile([128, 128], bf16, tag="tr", bufs=2)
                    nc.tensor.transpose(ps[:, :], genc[:, i, dsl], ident[:, :])
                    nc.vector.tensor_copy(gencT[dch][:, i * 128 : (i + 1) * 128], ps[:, :])

            # P_dec [52,640] bf16 -> SBUF->SBUF DMA relayout to row tile at
            # partitions 0/32/64
            pd_bf = persist.tile([3 * RG, V], bf16, tag="pd")
            nc.gpsimd.memset(pd_bf[:, :], 0.0)  # rows 52-53 stay zero (pad)
            ps = psum.tile([128, V], f32, tag="bc", bufs=3)
            for v0, vn in vchunks:
                for d in range(4):
                    nc.tensor.matmul(
                        ps[:UCORE, v0 : v0 + vn],
                        gdecT[d][:, :UCORE],
                        w_bf[:, 4 + d, v0 : v0 + vn],
                        start=(d == 0),
                        stop=(d == 3),
                    )
            nc.vector.tensor_copy(pd_bf[:UCORE, :], ps[:UCORE, :])
            rows = persist.tile([65, RG * V], bf16, tag="rows")
            nc.scalar.dma_start(rows[0:65:32, :], pd_bf[:, :])

            # P_enc (with bias): t-chunks 0,1 packed in pe2. The 44-row t-tail
            # (t 256-299) is computed TWICE: once landing on partitions 0-43
            # (pe3a) and once on partitions 64-107 (pe3b, lhsT cols 192-300
            # with rows 192-255 duplicated). The tail store alternates between
            # them per block parity so its bytes split between the even
            # (p0-63) and odd (p64-127) SDMA engine groups — otherwise the
            # even engines carry all tail descriptors and bound the loop.
            pe2 = persist.tile([128, 2, V], f32, tag="pe2")
            pe3a = persist.tile([44, V], f32, tag="pe3a")
            pe3b = persist.tile([108, V], f32, tag="pe3b")
            for i, (t0, tn) in enumerate([(0, 128), (128, 128), (256, 44), (192, 108)]):
                ps = psum.tile([128, V], f32, tag="bc", bufs=3)
                for v0, vn in vchunks:
                    for d in range(4):
                        nc.tensor.matmul(
                            ps[:tn, v0 : v0 + vn],
                            gencT[d][:, t0 : t0 + tn],
                            w_bf[:, d, v0 : v0 + vn],
                            start=(d == 0),
                            stop=False,
                        )
                    nc.tensor.matmul(
                        ps[:tn, v0 : v0 + vn],
                        ones3[0:1, :tn],
                        bias_sb[:1, v0 : v0 + vn],
                        start=False,
                        stop=True,
                    )
                if i < 2:
                    nc.scalar.copy(pe2[:, i, :], ps[:tn, :])
                elif i == 2:
                    nc.scalar.copy(pe3a[:, :], ps[:44, :])
                else:
                    nc.scalar.copy(pe3b[64:108, :], ps[64:108, :])

            # main loop: 13 blocks of 4 u
            for bi, u0 in enumerate(range(0, UCORE, UB)):
                par = bi % 2
                if par == 0:
                    bct4 = bctp.tile([44, UB, V], f32, tag="bcta", name="bct4a", bufs=2)
                    otB = outpB.tile([44, UB, V], f32, tag="otBa", name="otBa", bufs=2)
                    psl, pe3s = slice(0, 44), pe3a[:, :]
                else:
                    bct4 = bctp.tile([108, UB, V], f32, tag="bctb", name="bct4b", bufs=2)
                    otB = outpB.tile([108, UB, V], f32, tag="otBb", name="otBb", bufs=2)
                    psl, pe3s = slice(64, 108), pe3b[64:108, :]
                otA = outpA.tile([128, 2, UB, V], f32, tag="otA", name="otA")
                for j in range(UB):
                    u = u0 + j
                    g, off = u // RG, (u % RG) * V
                    ps = psum.tile([128, V], f32, tag="bc", bufs=3)
                    for c0, cn in ((0, 512), (512, V - 512)):
                        nc.tensor.matmul(
                            ps[:, c0 : c0 + cn],
                            ones3[32 * g : 32 * g + 1, :128],
                            rows[32 * g : 32 * g + 1, off + c0 : off + c0 + cn],
                            start=True,
                            stop=True,
                        )
                    nc.scalar.copy(bct4[psl, j, :], ps[psl, :])
                    nc.vector.tensor_add(
                        otA[:, :, j, :],
                        pe2[:, :, :],
                        ps[:, :].unsqueeze(1).broadcast_to([128, 2, V]),
                    )
                nc.gpsimd.tensor_add(
                    otB[psl, :, :],
                    pe3s.unsqueeze(1).broadcast_to([44, UB, V]),
                    bct4[psl, :, :],
                )
                engA, engB = (nc.sync, nc.scalar) if bi % 2 == 0 else (nc.scalar, nc.sync)
                if bi == 0:
                    # first block: store per u-pair across both rings so the
                    # first store issues right after the second DVE add
                    engA.dma_start(outA_d[bi, :, :, 0:2, :], otA[:, :, 0:2, :])
                    engB.dma_start(outA_d[bi, :, :, 2:4, :], otA[:, :, 2:4, :])
                    engA.dma_start(outB_d[bi, :, :, :], otB[psl, :, :])
                elif bi < UCORE // UB - 1:
                    engA.dma_start(outA_d[bi, :, :, :, :], otA[:, :, :, :])
                    engB.dma_start(outB_d[bi, :, :, :], otB[psl, :, :])
                else:
                    # last block: split the big store across both rings so the
                    # drain is half as long
                    engA.dma_start(outA_d[bi, :, 0, :, :], otA[:, 0, :, :])
                    engB.dma_start(outA_d[bi, :, 1, :, :], otA[:, 1, :, :])
                    engA.dma_start(outB_d[bi, :, :, :], otB[psl, :, :])

    nc.compile()
    return nc


def kernel(encoder_outputs, decoder_outputs, W_fc, b_fc):
    global LAST_RESULT
    import os

    import ml_dtypes
    from concourse.bass_utils import run_bass_kernel_spmd

    bf = ml_dtypes.bfloat16
    enc = np.asarray(encoder_outputs, dtype=np.float32)
    dec = np.asarray(decoder_outputs, dtype=np.float32)

    # enc per batch -> [128, 3, 512] with row t = c*128 + p, zero padded
    enc_pad = np.zeros((B, 384, D), dtype=np.float32)
    enc_pad[:, :T, :] = enc
    enc_tiled = np.ascontiguousarray(
        enc_pad.reshape(B, 3, 128, D).transpose(0, 2, 1, 3)
    ).astype(bf)

    # W_fc.T -> [128, 8, 640] with row d = c*128 + p
    wT = np.asarray(W_fc, dtype=np.float32).T  # (1024, 640)
    wT_tiled = np.ascontiguousarray(
        wT.reshape(8, 128, V).transpose(1, 0, 2)
    ).astype(bf)

    bias = np.asarray(b_fc, dtype=np.float32)[None, :].astype(bf)

    dec_pad = np.zeros((B, 2 * UCORE, D), dtype=np.float32)
    dec_pad[:, :U, :] = dec
    dec_pad = dec_pad.astype(bf)

    if "nc" not in _cache:
        _cache["nc"] = _build()
    nc = _cache["nc"]

    in_maps = []
    for c in range(NCORES):
        b, uh = c // 2, c % 2
        in_maps.append(
            {
                "enc": enc_tiled[b],
                "dec": np.ascontiguousarray(dec_pad[b, uh * UCORE : (uh + 1) * UCORE]),
                "wT": wT_tiled,
                "bias": bias,
            }
        )

    res = run_bass_kernel_spmd(
        nc,
        in_maps,
        list(range(NCORES)),
        trace=bool(int(os.environ.get("KJ_TRACE", "0"))),
        **RUN_KWARGS,
    )
    LAST_RESULT = res

    out = np.empty((B, T, U, V), dtype=np.float32)
    for c in range(NCORES):
        b, uh = c // 2, c % 2
        # outA (13,128,2,4,640): [bi,p,cc,j,v] -> t = cc*128+p, u = 4*bi+j
        # outB (13,44,4,640):    [bi,p,j,v]    -> t = 256+p,    u = 4*bi+j
        outA = res.results[c]["outA"]
        outB = res.results[c]["outB"]
        cut = np.empty((T, UCORE, V), dtype=np.float32)
        cut[:256] = outA.transpose(2, 1, 0, 3, 4).reshape(256, UCORE, V)
        cut[256:] = outB.transpose(1, 0, 2, 3).reshape(44, UCORE, V)
        if uh == 0:
            out[b, :, :UCORE] = cut
        else:
            out[b, :, UCORE:U] = cut[:, : U - UCORE]
    return out
